# revision 5
# baseline (speedup 1.0000x reference)
"""Trainium2 Bass kernel for nn_ConvDY2d (dynamic-weight 3x3 conv, CondConv-style).

Reference computation (B=16, C=O=256, H=W=64, K=4 mixing kernels):
  attn  = softmax(MLP(global_avg_pool(x)) / 30)            # [B, 4]
  w_mix = einsum('bk,koihw->boihw', attn, w_dyn)           # per-sample 3x3 conv kernel
  out[b] = conv2d(x[b], w_mix[b], padding=1)

Strategy: data-parallel over batch, 2 samples per NeuronCore across 8 cores.
Per core, the conv is an implicit GEMM: for each (out-channel block, 8-row
group) a [128, 512] PSUM tile accumulates 18 float matmuls (2 c-blocks x
9 taps) whose rhs are contiguous 512-element slices of a row-padded input
image ([128c, 4226]: 66 rows x 64 cols + 1 elem pad on each end).  Column
wrap-around at row edges is fixed up afterwards by subtracting per-border
correction terms computed with 12 small strided-rhs matmuls per output block.

Startup-latency-optimized schedule:
  - x[b0] loads first (fine chunks), pooling partials chase the DMA chunks
    (DVE on cb0, ACT on cb1) and are emitted BEFORE the wdyn DMAs so no
    stale-semaphore false deps delay them.
  - wdyn is loaded split by (k, cblock, pos-half) so the first mix chunk
    only waits for 1.25MB of DMA instead of the full 4.5MB.
  - weight mixing runs as all-bf16 scalar_tensor_tensor chains on DVE
    (fast 2-byte DVE mode), in-place into the bf16 wmix tile.
  - dummy matmuls keep the PE p-state ramped up until the first conv MM.
  - the conv per sample runs in PSUM groups of 5 tiles with 3 passes
    (cb0 pos0-4 / cb0 pos5-8 / cb1) so conv matmuls start as soon as the
    first quarter of the mixed weights exists.
"""

import sys

if "/opt/trn_rl_repo" not in sys.path:
    sys.path.insert(0, "/opt/trn_rl_repo")

import numpy as np

B, C, H, W = 16, 256, 64, 64
O, K, KS = 256, 4, 3
MID = C // 4
INV_DELTA = 1.0 / 30.0
NCORES = 8
NB = B // NCORES            # samples per core
NPOS = KS * KS              # 9 taps
FPAD = 1 + 66 * W + 1       # padded image free size: 4226
ROW0 = 65                   # flat offset of input row 0 (= 1 + 1*64)
HALF = 5 * O                # wmix free offset of pos-5 (first "half" = pos 0-4)

# x chunk row-splits per c-block: small final chunk so the last pooling
# partial is cheap and attention comes off the critical path.
XCHUNKS = [(0, 20), (20, 20), (40, 20), (60, 4)]

WARM1 = 32                  # [128,512] warm matmuls before the MLP
WARM2 = 40                  # [128,128] warm matmuls between MLP and conv

_CACHE = {}


def _build_nc():
    import concourse.bacc as bacc
    import concourse.tile as tile
    from concourse import mybir
    from concourse.tile_rust import add_dep_helper

    f32 = mybir.dt.float32
    bf16 = mybir.dt.bfloat16
    AX = mybir.AxisListType
    ALU = mybir.AluOpType
    ACTF = mybir.ActivationFunctionType

    nc = bacc.Bacc(target_bir_lowering=False, debug=False)

    x_d = nc.dram_tensor("x", [NB, C, H, W], bf16, kind="ExternalInput").ap()
    wd_d = nc.dram_tensor("wdynT", [K, NPOS, C, O], bf16, kind="ExternalInput").ap()
    fc1wT_d = nc.dram_tensor("fc1wT", [C, MID], f32, kind="ExternalInput").ap()
    fc1b_d = nc.dram_tensor("fc1b", [1, MID], f32, kind="ExternalInput").ap()
    fc2aug_d = nc.dram_tensor("fc2aug", [MID + 1, K], f32, kind="ExternalInput").ap()
    out_d = nc.dram_tensor("out", [NB, O, H, W], f32, kind="ExternalOutput").ap()

    with tile.TileContext(nc) as tc:
        with (
            tc.tile_pool(name="consts", bufs=1) as constp,
            tc.tile_pool(name="wdyn", bufs=1) as wdynp,
            tc.tile_pool(name="wmix", bufs=1) as wmixp,
            tc.tile_pool(name="xpad", bufs=1) as xpadp,
            tc.tile_pool(name="osb", bufs=6) as osbp,
            tc.tile_pool(name="convps", bufs=5, space="PSUM") as convps,
            tc.tile_pool(name="corrps", bufs=2, space="PSUM") as corrps,
            tc.tile_pool(name="smallps", bufs=1, space="PSUM") as smallps,
        ):
            # ---------------- tiny consts FIRST (ahead of bulk DMA) ----------
            fc1wT_sb = constp.tile([128, 2 * MID], f32, tag="fc1w", name="fc1wT_sb")
            for cb in range(2):
                nc.sync.dma_start(
                    fc1wT_sb[:, cb * MID : (cb + 1) * MID],
                    fc1wT_d[cb * 128 : (cb + 1) * 128, :],
                )
            fc1b_sb = constp.tile([1, MID], f32, tag="fc1b", name="fc1b_sb")
            nc.sync.dma_start(fc1b_sb, fc1b_d)
            fc2aug_sb = constp.tile([MID + 1, K], f32, tag="fc2", name="fc2aug_sb")
            nc.sync.dma_start(fc2aug_sb, fc2aug_d)

            ones_sb = constp.tile([1, 128], f32, tag="ones", name="ones_sb")
            nc.gpsimd.memset(ones_sb, 1.0)
            zero_h = constp.tile([128, HALF], bf16, tag="zeroh", name="zero_h")
            nc.gpsimd.memset(zero_h, 0.0)
            warm_rhs = constp.tile([128, 512], bf16, tag="warmr", name="warm_rhs")
            nc.gpsimd.memset(warm_rhs, 0.0)
            act_dummy = constp.tile([128, 20 * W], bf16, tag="actdum", name="act_dummy")

            # xpad tiles + pad memsets for both samples up front (gpsimd idle)
            xpad = [[None, None] for _ in range(NB)]
            for b in range(NB):
                for cb in range(2):
                    t = xpadp.tile([128, FPAD], bf16, tag=f"xpad{b}{cb}", name=f"xpad{b}{cb}")
                    nc.gpsimd.memset(t[:, 0:ROW0], 0.0)
                    nc.gpsimd.memset(t[:, ROW0 + H * W : FPAD], 0.0)
                    xpad[b][cb] = t

            def load_x(b):
                # interleave c-block chunks so both pooling engines get data
                for r0, nr in XCHUNKS:
                    for cb in range(2):
                        nc.sync.dma_start(
                            xpad[b][cb][:, ROW0 + r0 * W : ROW0 + (r0 + nr) * W],
                            x_d[
                                b, cb * 128 : (cb + 1) * 128, r0 : r0 + nr, :
                            ].rearrange("c h w -> c (h w)"),
                        )

            load_x(0)

            # ---------------- pooling + attention (sample 0) -----------------
            # Emitted BEFORE the wdyn DMAs: the pool partial reduces then wait
            # on exactly the x-chunk semaphore counts, with no false deps on
            # later DMAs that recycle the same semaphores.
            def pool_sample(b, engines):
                """Partial-sum pools chasing the x DMA chunks.
                engines: per-cb engine choice, 'dve' or 'act'."""
                pooled = [None, None]
                nch = len(XCHUNKS)
                for cb in range(2):
                    pp = constp.tile([128, nch], f32, tag=f"pp{b}{cb}", name=f"pp{b}{cb}")
                    for h, (r0, nr) in enumerate(XCHUNKS):
                        src = xpad[b][cb][:, ROW0 + r0 * W : ROW0 + (r0 + nr) * W]
                        if engines[cb] == "dve":
                            nc.vector.reduce_sum(pp[:, h : h + 1], src, AX.X)
                        else:
                            nc.scalar.activation(
                                act_dummy[:, 0 : nr * W], src, ACTF.Copy,
                                accum_out=pp[:, h : h + 1],
                            )
                    p = constp.tile([128, 1], f32, tag=f"pool{b}{cb}", name=f"pooled{b}{cb}")
                    nc.vector.reduce_sum(p, pp, AX.X)
                    pooled[cb] = p
                return pooled

            def attn_mlp(b, pooled, first_dep):
                """MLP + softmax + broadcast. Returns (attn_bc, first_mm, last_mm)."""
                hid_ps = smallps.tile([MID, 1], f32, tag="small", name=f"hid_ps{b}")
                mms = []
                for cb in range(2):
                    mm = nc.tensor.matmul(
                        hid_ps,
                        fc1wT_sb[:, cb * MID : (cb + 1) * MID],
                        pooled[cb],
                        start=(cb == 0),
                        stop=False,
                    )
                    mms.append(mm)
                if first_dep is not None:
                    add_dep_helper(mms[0].ins, first_dep.ins, sync=False,
                                   reason="PE order: warm before MLP")
                mms.append(nc.tensor.matmul(
                    hid_ps, fc1b_sb, ones_sb[:, 0:1], start=False, stop=True
                ))

                hid_sb = constp.tile([MID + 1, 1], f32, tag=f"hid{b}", name=f"hid_sb{b}")
                nc.gpsimd.memset(hid_sb[MID : MID + 1, :], 1.0)
                nc.scalar.activation(hid_sb[0:MID, :], hid_ps, ACTF.Relu)

                lg_ps = smallps.tile([1, K], f32, tag="small", name=f"lg_ps{b}")
                lg_mm = nc.tensor.matmul(lg_ps, hid_sb, fc2aug_sb, start=True, stop=True)

                ex = constp.tile([1, K], f32, tag=f"ex{b}", name=f"ex{b}")
                sm = constp.tile([1, 1], f32, tag=f"sm{b}", name=f"sm{b}")
                nc.scalar.activation(ex, lg_ps, ACTF.Exp, accum_out=sm)
                rc = constp.tile([1, 1], f32, tag=f"rc{b}", name=f"rc{b}")
                nc.vector.reciprocal(rc, sm)
                attn = constp.tile([1, K], f32, tag=f"at{b}", name=f"attn{b}")
                nc.vector.tensor_scalar_mul(attn, ex, rc)
                attn_bc = constp.tile([128, K], f32, tag=f"abc{b}", name=f"attn_bc{b}")
                nc.gpsimd.partition_broadcast(attn_bc, attn)
                return attn_bc, mms[0], lg_mm

            pooled0 = pool_sample(0, ("dve", "act"))

            # ---------------- PE warm stream part 1 --------------------------
            # Keeps the PE p-state ramped while DMAs/pooling run. Chained WAW
            # on one corrps bank keeps them in order.
            warm_ps = corrps.tile([128, 512], f32, tag="corr", name="warm_ps1")
            warm1_last = None
            for i in range(WARM1):
                warm1_last = nc.tensor.matmul(
                    warm_ps, warm_rhs[:, 0:128], warm_rhs, start=True, stop=True
                )

            attn_bc0, mlp0_first, mlp0_last = attn_mlp(0, pooled0, warm1_last)

            # ---------------- PE warm stream part 2 (finer grain) ------------
            warm_ps2 = corrps.tile([128, 512], f32, tag="corr", name="warm_ps2")
            warm2_last = None
            for i in range(WARM2):
                mm = nc.tensor.matmul(
                    warm_ps2[:, 0:128], warm_rhs[:, 0:128], warm_rhs[:, 0:128],
                    start=True, stop=True,
                )
                if i == 0:
                    add_dep_helper(mm.ins, mlp0_last.ins, sync=False,
                                   reason="PE order: MLP before warm2")
                warm2_last = mm

            # ---------------- wdyn loads, split by (cb, pos-half, k) ---------
            # First 4 DMAs (cb0 pos0-4, all k) are all the first mix chunk
            # needs -> conv matmuls can start ~1.25MB into the wdyn load.
            wdyn = [[None, None] for _ in range(K)]
            for cb in range(2):
                for k in range(K):
                    wdyn[k][cb] = wdynp.tile(
                        [128, NPOS * O], bf16, tag=f"wd{k}{cb}", name=f"wd{k}{cb}"
                    )
            for cb in range(2):
                for p0, p1 in ((0, 5), (5, NPOS)):
                    for k in range(K):
                        nc.sync.dma_start(
                            wdyn[k][cb].rearrange("c (p o) -> c p o", o=O)[:, p0:p1, :],
                            wd_d[k, p0:p1, cb * 128 : (cb + 1) * 128, :].transpose([1, 0, 2]),
                        )

            # ---------------- weight mixing (all-bf16 DVE chains) ------------
            wmix = [[None, None] for _ in range(NB)]
            mix_last = [None]  # threaded ordering across all chains

            def mix_sample(b, attn_bc):
                for cb in range(2):
                    wm = wmixp.tile(
                        [128, NPOS * O], bf16, tag=f"wm{b}{cb}", name=f"wmix{b}{cb}"
                    )
                    for lo, hi in ((0, HALF), (HALF, NPOS * O)):
                        first = nc.vector.scalar_tensor_tensor(
                            wm[:, lo:hi],
                            wdyn[0][cb][:, lo:hi],
                            attn_bc[:, 0:1],
                            zero_h[:, 0 : hi - lo],
                            op0=ALU.mult,
                            op1=ALU.add,
                        )
                        if mix_last[0] is not None:
                            add_dep_helper(first.ins, mix_last[0].ins, sync=False,
                                           reason="mix chain ordering")
                        last = first
                        for k in range(1, K):
                            last = nc.vector.scalar_tensor_tensor(
                                wm[:, lo:hi],
                                wdyn[k][cb][:, lo:hi],
                                attn_bc[:, k : k + 1],
                                wm[:, lo:hi],
                                op0=ALU.mult,
                                op1=ALU.add,
                            )
                        mix_last[0] = last
                    wmix[b][cb] = wm

            mix_sample(0, attn_bc0)

            # x[1] DMAs queue right behind wdyn on the DMA engines
            load_x(1)

            # ---------------- conv ------------------------------------------
            def wsl(b, cb, pos, ob):
                off = pos * O + ob * 128
                return wmix[b][cb][:, off : off + 128]

            TILES = [(ob, rg) for ob in range(2) for rg in range(8)]
            GROUPS = [TILES[0:5], TILES[5:10], TILES[10:15], TILES[15:16]]

            def corr_block(b, ob, dep_mm):
                corr = corrps.tile([128, 128], f32, tag="corr", name=f"corr{b}{ob}")
                first = True
                for side, dxv in ((0, 0), (1, 2)):
                    i = 0
                    for cb in range(2):
                        for dy in range(KS):
                            s = dy * W + (0 if side == 0 else ROW0)
                            rhs = xpad[b][cb][:, s : s + (H - 1) * W + 1 : W]
                            mm = nc.tensor.matmul(
                                corr[:, side * 64 : side * 64 + 64],
                                wsl(b, cb, dy * KS + dxv, ob),
                                rhs,
                                start=(i == 0),
                                stop=(i == 5),
                            )
                            if first and dep_mm is not None:
                                add_dep_helper(mm.ins, dep_mm.ins, sync=False,
                                               reason="PE order: corr after C0")
                            first = False
                            i += 1
                return corr

            def conv_sample(b, first_dep):
                corr = {}
                dep = first_dep
                for gi, group in enumerate(GROUPS):
                    cps = {}
                    for ob, rg in group:
                        cps[(ob, rg)] = convps.tile(
                            [128, 512], f32, tag="conv", name=f"cps{b}{ob}{rg}"
                        )
                    last_mm = None

                    # pass A+B: cb0 pos0-4 then pos5-8; pass C0: cb1 pos0-4
                    for cb, prange in ((0, range(0, 5)), (0, range(5, 9)), (1, range(0, 5))):
                        for pos in prange:
                            dy, dx = divmod(pos, 3)
                            for ob, rg in group:
                                s = (rg * 8 + dy) * W + dx
                                mm = nc.tensor.matmul(
                                    cps[(ob, rg)],
                                    wsl(b, cb, pos, ob),
                                    xpad[b][cb][:, s : s + 512],
                                    start=(cb == 0 and pos == 0),
                                    stop=False,
                                )
                                if dep is not None:
                                    add_dep_helper(mm.ins, dep.ins, sync=False,
                                                   reason="PE phase order")
                                    dep = None
                                last_mm = mm

                    # border corrections for each ob first seen in this group
                    for ob in {ob for ob, _ in group}:
                        if (b, ob) not in corr:
                            corr[(b, ob)] = corr_block(b, ob, last_mm)

                    # pass C1: cb1 pos5-8, tile-major so early tiles finish
                    # (and free PSUM banks) before the pass ends
                    for ob, rg in group:
                        for pos in range(5, 9):
                            dy, dx = divmod(pos, 3)
                            s = (rg * 8 + dy) * W + dx
                            last_mm = nc.tensor.matmul(
                                cps[(ob, rg)],
                                wsl(b, 1, pos, ob),
                                xpad[b][1][:, s : s + 512],
                                start=False,
                                stop=(pos == 8),
                            )

                    for ob, rg in group:
                        y0 = rg * 8
                        osb = osbp.tile([128, 512], f32, tag="osb", name=f"osb{b}{ob}{rg}")
                        nc.scalar.copy(osb, cps[(ob, rg)])
                        ov = osb.rearrange("m (y x) -> m y x", x=W)[:, :, 0 : W : W - 1]
                        cv = corr[(b, ob)].rearrange("m (s y) -> m y s", s=2)[:, y0 : y0 + 8, :]
                        nc.vector.tensor_sub(ov, ov, cv)
                        nc.sync.dma_start(
                            out_d[b, ob * 128 : (ob + 1) * 128, y0 : y0 + 8, :],
                            osb.rearrange("m (y x) -> m y x", x=W),
                        )
                    yield last_mm

            g0 = conv_sample(0, warm2_last)
            next(g0)  # G1
            g2_last = next(g0)  # G2

            # sample-1 attention: pools on ACT (DVE is busy mixing), MLP MMs
            # pinned behind b0's G2 on the PE stream so they never stall it.
            pooled1 = pool_sample(1, ("act", "act"))
            attn_bc1, mlp1_first, _ = attn_mlp(1, pooled1, g2_last)
            mix_sample(1, attn_bc1)

            for _ in g0:  # G3, G4
                pass
            for _ in conv_sample(1, None):
                pass

    nc.compile()
    return nc


def get_nc():
    if "nc" not in _CACHE:
        _CACHE["nc"] = _build_nc()
    return _CACHE["nc"]


def prep_inputs(x, w_dyn, fc1_w, fc1_b, fc2_w, fc2_b):
    """Host-side layout prep + batch sharding -> per-core input maps."""
    import ml_dtypes

    bf16 = ml_dtypes.bfloat16
    w_dynT = np.ascontiguousarray(
        np.transpose(np.asarray(w_dyn, np.float32), (0, 3, 4, 2, 1)).reshape(K, NPOS, C, O)
    ).astype(bf16)
    fc1wT = np.ascontiguousarray(np.asarray(fc1_w, np.float32).T) / float(H * W)
    fc1b = np.ascontiguousarray(np.asarray(fc1_b, np.float32).reshape(1, MID))
    fc2aug = np.ascontiguousarray(
        np.vstack([np.asarray(fc2_w, np.float32).T, np.asarray(fc2_b, np.float32)[None, :]])
        * INV_DELTA
    )
    x = np.asarray(x, np.float32).astype(bf16)
    in_maps = []
    for core in range(NCORES):
        in_maps.append(
            {
                "x": np.ascontiguousarray(x[core * NB : (core + 1) * NB]),
                "wdynT": w_dynT,
                "fc1wT": fc1wT,
                "fc1b": fc1b,
                "fc2aug": fc2aug,
            }
        )
    return in_maps


def kernel(x, w_dyn, fc1_w, fc1_b, fc2_w, fc2_b):
    from concourse.bass_utils import run_bass_kernel_spmd

    nc = get_nc()
    in_maps = prep_inputs(x, w_dyn, fc1_w, fc1_b, fc2_w, fc2_b)
    res = run_bass_kernel_spmd(nc, in_maps, core_ids=list(range(NCORES)))
    return np.concatenate([r["out"] for r in res.results], axis=0)


# revision 7
# speedup vs baseline: 1.0265x; 1.0265x over previous
"""Trainium2 Bass kernel for nn_ConvDY2d (dynamic-weight 3x3 conv, CondConv-style).

Reference computation (B=16, C=O=256, H=W=64, K=4 mixing kernels):
  attn  = softmax(MLP(global_avg_pool(x)) / 30)            # [B, 4]
  w_mix = einsum('bk,koihw->boihw', attn, w_dyn)           # per-sample 3x3 conv kernel
  out[b] = conv2d(x[b], w_mix[b], padding=1)

Strategy: data-parallel over batch, 2 samples per NeuronCore across 8 cores.
Per core, the conv is an implicit GEMM: for each (out-channel block, 8-row
group) a [128, 512] PSUM tile accumulates 18 matmuls (2 c-blocks x 9 taps)
whose rhs are contiguous 512-element slices of a row-padded input image
([128c, 4226]).  Column wrap-around at row edges is fixed up afterwards by
subtracting border corrections computed with 12 strided-rhs matmuls per
output block.

Startup-latency-optimized schedule:
  - x[b0] loads first (row chunks); pooling partials chase the DMA chunks
    (DVE cb0 / ACT cb1), dep-chained in order so the tile scheduler cannot
    reorder them behind later DMAs that recycle the same semaphores.
  - wdyn is loaded split by (cblock, dy-row, k): the first mix chunk only
    needs 0.75MB of wdyn, so conv matmuls start ~19us instead of ~32us.
  - weight mixing uses tensor_scalar (4x DVE mode) + tensor_tensor (2x)
    trees in bf16: 7 ops/chunk instead of a 4-long scalar_tensor_tensor
    chain at 1x.
  - the conv runs in PSUM groups of 5 tiles with per-(cb,dy) passes so the
    matmul stream chases the mix chunks without stalling.
"""

import sys

if "/opt/trn_rl_repo" not in sys.path:
    sys.path.insert(0, "/opt/trn_rl_repo")

import numpy as np

B, C, H, W = 16, 256, 64, 64
O, K, KS = 256, 4, 3
MID = C // 4
INV_DELTA = 1.0 / 30.0
NCORES = 8
NB = B // NCORES            # samples per core
NPOS = KS * KS              # 9 taps
FPAD = 1 + 66 * W + 1       # padded image free size: 4226
ROW0 = 65                   # flat offset of input row 0 (= 1 + 1*64)

# x chunk row-splits per c-block: small final chunk so the last pooling
# partial is cheap and attention comes off the critical path.
XCHUNKS = [(0, 20), (20, 20), (40, 20), (60, 4)]

_CACHE = {}


def _build_nc():
    import concourse.bacc as bacc
    import concourse.tile as tile
    from concourse import mybir
    from concourse.tile_rust import add_dep_helper

    f32 = mybir.dt.float32
    bf16 = mybir.dt.bfloat16
    AX = mybir.AxisListType
    ALU = mybir.AluOpType
    ACTF = mybir.ActivationFunctionType

    nc = bacc.Bacc(target_bir_lowering=False, debug=False)

    x_d = nc.dram_tensor("x", [NB, C, H, W], bf16, kind="ExternalInput").ap()
    wd_d = nc.dram_tensor("wdynT", [K, NPOS, C, O], bf16, kind="ExternalInput").ap()
    fc1wT_d = nc.dram_tensor("fc1wT", [C, MID], f32, kind="ExternalInput").ap()
    fc1b_d = nc.dram_tensor("fc1b", [1, MID], f32, kind="ExternalInput").ap()
    fc2aug_d = nc.dram_tensor("fc2aug", [MID + 1, K], f32, kind="ExternalInput").ap()
    out_d = nc.dram_tensor("out", [NB, O, H, W], f32, kind="ExternalOutput").ap()

    with tile.TileContext(nc) as tc:
        with (
            tc.tile_pool(name="consts", bufs=1) as constp,
            tc.tile_pool(name="wdyn", bufs=1) as wdynp,
            tc.tile_pool(name="wmix", bufs=1) as wmixp,
            tc.tile_pool(name="xpad", bufs=1) as xpadp,
            tc.tile_pool(name="osb", bufs=6) as osbp,
            tc.tile_pool(name="convps", bufs=5, space="PSUM") as convps,
            tc.tile_pool(name="corrps", bufs=2, space="PSUM") as corrps,
            tc.tile_pool(name="smallps", bufs=1, space="PSUM") as smallps,
        ):
            # ---------------- tiny consts FIRST (ahead of bulk DMA) ----------
            fc1wT_sb = constp.tile([128, 2 * MID], f32, tag="fc1w", name="fc1wT_sb")
            for cb in range(2):
                nc.sync.dma_start(
                    fc1wT_sb[:, cb * MID : (cb + 1) * MID],
                    fc1wT_d[cb * 128 : (cb + 1) * 128, :],
                )
            fc1b_sb = constp.tile([1, MID], f32, tag="fc1b", name="fc1b_sb")
            nc.sync.dma_start(fc1b_sb, fc1b_d)
            fc2aug_sb = constp.tile([MID + 1, K], f32, tag="fc2", name="fc2aug_sb")
            nc.sync.dma_start(fc2aug_sb, fc2aug_d)

            ones_sb = constp.tile([1, 128], f32, tag="ones", name="ones_sb")
            nc.gpsimd.memset(ones_sb, 1.0)
            act_dummy = constp.tile([128, 20 * W], bf16, tag="actdum", name="act_dummy")

            # xpad tiles + pad memsets for both samples up front (gpsimd idle)
            xpad = [[None, None] for _ in range(NB)]
            for b in range(NB):
                for cb in range(2):
                    t = xpadp.tile([128, FPAD], bf16, tag=f"xpad{b}{cb}", name=f"xpad{b}{cb}")
                    nc.gpsimd.memset(t[:, 0:ROW0], 0.0)
                    nc.gpsimd.memset(t[:, ROW0 + H * W : FPAD], 0.0)
                    xpad[b][cb] = t

            def load_x(b):
                # interleave c-block chunks so both pooling engines get data
                for r0, nr in XCHUNKS:
                    for cb in range(2):
                        nc.sync.dma_start(
                            xpad[b][cb][:, ROW0 + r0 * W : ROW0 + (r0 + nr) * W],
                            x_d[
                                b, cb * 128 : (cb + 1) * 128, r0 : r0 + nr, :
                            ].rearrange("c h w -> c (h w)"),
                        )

            load_x(0)

            # ---------------- pooling + attention (sample 0) -----------------
            # Partials are dep-chained in chunk order: the scheduler would
            # otherwise reorder them and regenerate semaphore targets against
            # later DMAs that recycle the same semaphores (false deps).
            def pool_sample(b, engines):
                pooled = [None, None]
                nch = len(XCHUNKS)
                prev = [None, None]
                for h, (r0, nr) in enumerate(XCHUNKS):
                    for cb in range(2):
                        if h == 0:
                            pp = constp.tile(
                                [128, nch], f32, tag=f"pp{b}{cb}", name=f"pp{b}{cb}"
                            )
                            if cb == 0:
                                pp0 = pp
                            else:
                                pp1 = pp
                        pp = pp0 if cb == 0 else pp1
                        src = xpad[b][cb][:, ROW0 + r0 * W : ROW0 + (r0 + nr) * W]
                        if engines[cb] == "dve":
                            r = nc.vector.reduce_sum(pp[:, h : h + 1], src, AX.X)
                        else:
                            r = nc.scalar.activation(
                                act_dummy[:, 0 : nr * W], src, ACTF.Copy,
                                accum_out=pp[:, h : h + 1],
                            )
                        if prev[cb] is not None:
                            add_dep_helper(r.ins, prev[cb].ins, sync=False,
                                           reason="pool chunk order")
                        prev[cb] = r
                for cb in range(2):
                    pp = pp0 if cb == 0 else pp1
                    p = constp.tile([128, 1], f32, tag=f"pool{b}{cb}", name=f"pooled{b}{cb}")
                    r = nc.vector.reduce_sum(p, pp, AX.X)
                    if engines[cb] == "dve":
                        add_dep_helper(r.ins, prev[cb].ins, sync=False,
                                       reason="pool final order")
                    pooled[cb] = p
                return pooled

            def attn_mlp(b, pooled, first_dep):
                hid_ps = smallps.tile([MID, 1], f32, tag="small", name=f"hid_ps{b}")
                first_mm = None
                for cb in range(2):
                    mm = nc.tensor.matmul(
                        hid_ps,
                        fc1wT_sb[:, cb * MID : (cb + 1) * MID],
                        pooled[cb],
                        start=(cb == 0),
                        stop=False,
                    )
                    if first_mm is None:
                        first_mm = mm
                        if first_dep is not None:
                            add_dep_helper(mm.ins, first_dep.ins, sync=False,
                                           reason="PE order for MLP")
                nc.tensor.matmul(hid_ps, fc1b_sb, ones_sb[:, 0:1], start=False, stop=True)

                hid_sb = constp.tile([MID + 1, 1], f32, tag=f"hid{b}", name=f"hid_sb{b}")
                nc.gpsimd.memset(hid_sb[MID : MID + 1, :], 1.0)
                nc.scalar.activation(hid_sb[0:MID, :], hid_ps, ACTF.Relu)

                lg_ps = smallps.tile([1, K], f32, tag="small", name=f"lg_ps{b}")
                nc.tensor.matmul(lg_ps, hid_sb, fc2aug_sb, start=True, stop=True)

                ex = constp.tile([1, K], f32, tag=f"ex{b}", name=f"ex{b}")
                sm = constp.tile([1, 1], f32, tag=f"sm{b}", name=f"sm{b}")
                nc.scalar.activation(ex, lg_ps, ACTF.Exp, accum_out=sm)
                rc = constp.tile([1, 1], f32, tag=f"rc{b}", name=f"rc{b}")
                nc.vector.reciprocal(rc, sm)
                attn = constp.tile([1, K], f32, tag=f"at{b}", name=f"attn{b}")
                nc.vector.tensor_scalar_mul(attn, ex, rc)
                attn_bc = constp.tile([128, K], f32, tag=f"abc{b}", name=f"attn_bc{b}")
                nc.gpsimd.partition_broadcast(attn_bc, attn)
                return attn_bc

            pooled0 = pool_sample(0, ("dve", "act"))
            attn_bc0 = attn_mlp(0, pooled0, None)

            # ---------------- wdyn loads, split by (cb, dy-row, k) -----------
            # First 4 DMAs (cb0 dy0, all k) are all the first mix chunk needs.
            wdyn = [[None, None] for _ in range(K)]
            for cb in range(2):
                for k in range(K):
                    wdyn[k][cb] = wdynp.tile(
                        [128, NPOS * O], bf16, tag=f"wd{k}{cb}", name=f"wd{k}{cb}"
                    )
            for cb in range(2):
                for dy in range(KS):
                    for k in range(K):
                        nc.sync.dma_start(
                            wdyn[k][cb].rearrange("c (p o) -> c p o", o=O)[
                                :, dy * KS : (dy + 1) * KS, :
                            ],
                            wd_d[
                                k, dy * KS : (dy + 1) * KS, cb * 128 : (cb + 1) * 128, :
                            ].transpose([1, 0, 2]),
                        )

            # ---------------- weight mixing: bf16 ts/tt trees on DVE ---------
            # Per (cb, dy) chunk of [128, 768]:
            #   wm = a0*w0; s1 = a1*w1; wm += s1; s1 = a2*w2; s2 = a3*w3;
            #   s1 += s2; wm += s1          (4x tensor_scalar, 2x tensor_tensor)
            mix_s1 = constp.tile([128, KS * O], bf16, tag="mixs1", name="mix_s1")
            mix_s2 = constp.tile([128, KS * O], bf16, tag="mixs2", name="mix_s2")
            wmix = [[None, None] for _ in range(NB)]
            mix_last = [None]

            def mix_sample(b, attn_bc):
                for cb in range(2):
                    wm = wmixp.tile(
                        [128, NPOS * O], bf16, tag=f"wm{b}{cb}", name=f"wmix{b}{cb}"
                    )
                    for dy in range(KS):
                        lo, hi = dy * KS * O, (dy + 1) * KS * O
                        wmh = wm[:, lo:hi]
                        first = nc.vector.tensor_scalar_mul(
                            wmh, wdyn[0][cb][:, lo:hi], attn_bc[:, 0:1]
                        )
                        if mix_last[0] is not None:
                            add_dep_helper(first.ins, mix_last[0].ins, sync=False,
                                           reason="mix chunk order")
                        nc.vector.tensor_scalar_mul(
                            mix_s1, wdyn[1][cb][:, lo:hi], attn_bc[:, 1:2]
                        )
                        nc.vector.tensor_tensor(wmh, wmh, mix_s1, op=ALU.add)
                        nc.vector.tensor_scalar_mul(
                            mix_s1, wdyn[2][cb][:, lo:hi], attn_bc[:, 2:3]
                        )
                        nc.vector.tensor_scalar_mul(
                            mix_s2, wdyn[3][cb][:, lo:hi], attn_bc[:, 3:4]
                        )
                        nc.vector.tensor_tensor(mix_s1, mix_s1, mix_s2, op=ALU.add)
                        mix_last[0] = nc.vector.tensor_tensor(
                            wmh, wmh, mix_s1, op=ALU.add
                        )
                    wmix[b][cb] = wm

            mix_sample(0, attn_bc0)

            # x[1] DMAs queue right behind wdyn on the DMA engines
            load_x(1)

            # ---------------- conv ------------------------------------------
            def wsl(b, cb, pos, ob):
                off = pos * O + ob * 128
                return wmix[b][cb][:, off : off + 128]

            TILES = [(ob, rg) for ob in range(2) for rg in range(8)]
            GROUPS = [TILES[0:5], TILES[5:10], TILES[10:15], TILES[15:16]]

            def corr_block(b, ob, dep_mm):
                corr = corrps.tile([128, 128], f32, tag="corr", name=f"corr{b}{ob}")
                first = True
                for side, dxv in ((0, 0), (1, 2)):
                    i = 0
                    for cb in range(2):
                        for dy in range(KS):
                            s = dy * W + (0 if side == 0 else ROW0)
                            rhs = xpad[b][cb][:, s : s + (H - 1) * W + 1 : W]
                            mm = nc.tensor.matmul(
                                corr[:, side * 64 : side * 64 + 64],
                                wsl(b, cb, dy * KS + dxv, ob),
                                rhs,
                                start=(i == 0),
                                stop=(i == 5),
                            )
                            if first and dep_mm is not None:
                                add_dep_helper(mm.ins, dep_mm.ins, sync=False,
                                               reason="PE order: corr")
                            first = False
                            i += 1
                return corr

            def conv_sample(b):
                corr = {}
                for group in GROUPS:
                    cps = {}
                    for ob, rg in group:
                        cps[(ob, rg)] = convps.tile(
                            [128, 512], f32, tag="conv", name=f"cps{b}{ob}{rg}"
                        )
                    last_mm = None

                    # 6 passes: (cb, dy) in mix-chunk production order; the
                    # last (cb1, dy2) pass is tile-major so tiles retire early.
                    for cb in range(2):
                        for dy in range(KS):
                            final = cb == 1 and dy == KS - 1
                            if final:
                                order = [
                                    (ob, rg, pos)
                                    for ob, rg in group
                                    for pos in range(dy * KS, dy * KS + KS)
                                ]
                            else:
                                order = [
                                    (ob, rg, pos)
                                    for pos in range(dy * KS, dy * KS + KS)
                                    for ob, rg in group
                                ]
                            for ob, rg, pos in order:
                                ddy, dx = divmod(pos, 3)
                                s = (rg * 8 + ddy) * W + dx
                                last_mm = nc.tensor.matmul(
                                    cps[(ob, rg)],
                                    wsl(b, cb, pos, ob),
                                    xpad[b][cb][:, s : s + 512],
                                    start=(cb == 0 and pos == 0),
                                    stop=(final and pos == NPOS - 1),
                                )
                            # border corrections once per ob, after cb1 dy0
                            # (all wmix ready by then, subs come later)
                            if cb == 1 and dy == 0:
                                for ob in sorted({ob for ob, _ in group}):
                                    if (b, ob) not in corr:
                                        corr[(b, ob)] = corr_block(b, ob, last_mm)

                    for ob, rg in group:
                        y0 = rg * 8
                        osb = osbp.tile([128, 512], f32, tag="osb", name=f"osb{b}{ob}{rg}")
                        nc.scalar.copy(osb, cps[(ob, rg)])
                        ov = osb.rearrange("m (y x) -> m y x", x=W)[:, :, 0 : W : W - 1]
                        cv = corr[(b, ob)].rearrange("m (s y) -> m y s", s=2)[:, y0 : y0 + 8, :]
                        nc.vector.tensor_sub(ov, ov, cv)
                        nc.sync.dma_start(
                            out_d[b, ob * 128 : (ob + 1) * 128, y0 : y0 + 8, :],
                            osb.rearrange("m (y x) -> m y x", x=W),
                        )
                    yield last_mm

            g0 = conv_sample(0)
            next(g0)  # G1
            g2_last = next(g0)  # G2

            # sample-1 attention: pools on ACT (DVE mixes), MLP matmuls pinned
            # behind b0's G2 on the PE stream so they never stall it.
            pooled1 = pool_sample(1, ("act", "act"))
            attn_bc1 = attn_mlp(1, pooled1, g2_last)
            mix_sample(1, attn_bc1)

            for _ in g0:  # G3, G4
                pass
            for _ in conv_sample(1):
                pass

    nc.compile()
    return nc


def get_nc():
    if "nc" not in _CACHE:
        _CACHE["nc"] = _build_nc()
    return _CACHE["nc"]


def prep_inputs(x, w_dyn, fc1_w, fc1_b, fc2_w, fc2_b):
    """Host-side layout prep + batch sharding -> per-core input maps."""
    import ml_dtypes

    bf16 = ml_dtypes.bfloat16
    w_dynT = np.ascontiguousarray(
        np.transpose(np.asarray(w_dyn, np.float32), (0, 3, 4, 2, 1)).reshape(K, NPOS, C, O)
    ).astype(bf16)
    fc1wT = np.ascontiguousarray(np.asarray(fc1_w, np.float32).T) / float(H * W)
    fc1b = np.ascontiguousarray(np.asarray(fc1_b, np.float32).reshape(1, MID))
    fc2aug = np.ascontiguousarray(
        np.vstack([np.asarray(fc2_w, np.float32).T, np.asarray(fc2_b, np.float32)[None, :]])
        * INV_DELTA
    )
    x = np.asarray(x, np.float32).astype(bf16)
    in_maps = []
    for core in range(NCORES):
        in_maps.append(
            {
                "x": np.ascontiguousarray(x[core * NB : (core + 1) * NB]),
                "wdynT": w_dynT,
                "fc1wT": fc1wT,
                "fc1b": fc1b,
                "fc2aug": fc2aug,
            }
        )
    return in_maps


def kernel(x, w_dyn, fc1_w, fc1_b, fc2_w, fc2_b):
    from concourse.bass_utils import run_bass_kernel_spmd

    nc = get_nc()
    in_maps = prep_inputs(x, w_dyn, fc1_w, fc1_b, fc2_w, fc2_b)
    res = run_bass_kernel_spmd(nc, in_maps, core_ids=list(range(NCORES)))
    return np.concatenate([r["out"] for r in res.results], axis=0)
